# revision 37
# baseline (speedup 1.0000x reference)
"""Trainium2 Bass kernel for nn_MinLoss_69707319214519.

Computes log(min_p mean_b |sum_s D[b,s,perm[p,s]]/3|) where
D[b,s,r] = ||P[b,:,s,:] - G[b,:,r,:]||_F over (seq, dim).

Strategy (8 cores, 2 batches/core, bf16 streaming, compute-balanced):
  Inputs are cast to bf16 on the host (free), halving HBM traffic; the
  cost model then makes the three compute engines the bottleneck.  The
  squared distances are accumulated DIRECTLY as D2[s,r] = sum (P_s-G_r)^2
  (no Gram decomposition), so each chunk needs only the 9 (s,r) pairs.
  Each pair takes one of three balanced paths:
    p1 : DVE tensor_tensor subtract (2x bf16 mode) -> ACT Square-accum
    p2 : DVE subtract + DVE self-mult (both 2x)    -> Pool XYZWC reduce
    p3 : Pool subtract                             -> ACT Square-accum
  Chunk sizes ramp [2,2,4,8,...] and every chunk streams per-source
  (p0,g0,p1,g1,p2,g2) with ops emitted in data-unlock order, so all
  engines start within ~5us. Path counts per chunk come from an LP that
  equalizes engine END times (pipeline lags included), error-diffused
  to integers.  Host: gather partial sums -> D -> perm sums -> log(min).
"""

import numpy as np

B = 16
T = 4096
S = 3
DIM = 512
N_CORES = 8
B_PER_CORE = B // N_CORES          # 2
P = 128                            # SBUF partitions
ROW = S * DIM                      # 1536

# per-batch chunk schedule (units of P seq rows); each batch sums to 32.
_SCHED = [[2, 2, 4, 8, 8, 8], [8, 8, 8, 8]]

CROSS = [(s, r) for s in range(S) for r in range(S)]

PERMS3 = np.array(
    [[0, 1, 2], [0, 2, 1], [1, 0, 2], [1, 2, 0], [2, 0, 1], [2, 1, 0]]
)


def _op_costs(n):
    """Cost-model engine-busy ns per op of free-size n (bf16, calibrated
    against TimelineSim traces of this kernel)."""
    tt = 61 + 0.5208 * n             # DVE tensor_tensor (2x_1p mode)
    act = 559 + 0.8333 * n           # ACT activation + 187ns accum read
    pool_red = 95 + 1.3889 * n       # Pool reduce_sum XYZWC
    pool_tt = 95 + 2.0700 * n        # Pool tensor_tensor
    return tt, act, pool_red, pool_tt


# engine pipeline start lags (ns, whole program), tuned on traces
_LAG_DVE = 5400.0
_LAG_ACT = 7000.0
_LAG_POOL = 9000.0
# last chunks carry no p2 (pool reduces are second-order consumers and
# would gate the program end on the DVE->pool chain)
_TAIL_NORED = 0
# optional explicit per-chunk (y, z) override, list of pairs or None
_YZ_OVERRIDE = None
# scheduler priority boost for the pool-feeding ops (p2/p3 chains), in
# program-order units; lets next-chunk feeds outrank p1 backlog
_FEED_PRIO = 15


def _lp_targets(n, nchunks):
    """Fractional (p1, p2, p3) pair counts equalizing engine END times."""
    tt, act, pr, pt = _op_costs(n)
    la_d = _LAG_DVE / nchunks
    la_a = _LAG_ACT / nchunks
    la_p = _LAG_POOL / nchunks
    # D = tt x + 2 tt y + la_d ; A = act (x+z) + la_a ; P = pr y + pt z + la_p
    mat = np.array(
        [
            [tt - act, 2 * tt, -act],
            [act, -pr, act - pt],
            [1.0, 1.0, 1.0],
        ]
    )
    rhs = np.array([la_a - la_d, la_p - la_a, 9.0])
    try:
        x, y, z = np.linalg.solve(mat, rhs)
    except np.linalg.LinAlgError:
        x, y, z = 6.0, 2.0, 1.0
    x, y, z = max(x, 0.0), max(y, 0.0), max(z, 0.0)
    scale = 9.0 / (x + y + z)
    return x * scale, y * scale, z * scale


def _chunk_plan():
    """Per-chunk path assignments with error-diffused integer counts.

    Returns list of dicts: b, u, p1[(col,s,r)], p2[(col,s,r)], p3[(col,s,r)]
    plus global column totals (ACT cols for p1/p3, Pool cols for p2).
    """
    plan = []
    col_a = col_p = 0
    acc_y = acc_z = 0.0
    rot = 0
    nchunks = sum(len(s) for s in _SCHED)
    ci = 0
    for b, sched in enumerate(_SCHED):
        for u in sched:
            n = u * DIM
            _, y_t, z_t = _lp_targets(n, nchunks)
            acc_y += y_t
            y_i = int(np.floor(acc_y + 0.5))
            acc_y -= y_i
            acc_z += z_t
            z_i = int(np.floor(acc_z + 0.5))
            acc_z -= z_i
            y_i = min(y_i, 9)
            z_i = min(z_i, 9 - y_i)
            if ci >= nchunks - _TAIL_NORED:
                y_i = 0          # tail p2 pairs fall through to p1
            if _YZ_OVERRIDE is not None:
                y_i, z_i = _YZ_OVERRIDE[ci]
            ci += 1
            # earliest-unlocking pairs (data arrives per source
            # p0,g0,p1,g1,p2,g2) go to the pool-queue-independent p3 subs,
            # then to the p2 feed chain, so no engine head-of-line blocks
            pairs = CROSS[rot:] + CROSS[:rot]
            pairs = sorted(pairs, key=lambda sr: max(2 * sr[0], 2 * sr[1] + 1))
            rot = (rot + 2) % 9
            d = dict(b=b, u=u, p1=[], p2=[], p3=[])
            for s, r in pairs[:z_i]:
                d["p3"].append((col_a, s, r))
                col_a += 1
            for s, r in pairs[z_i : z_i + y_i]:
                d["p2"].append((col_p, s, r))
                col_p += 1
            for s, r in pairs[z_i + y_i :]:
                d["p1"].append((col_a, s, r))
                col_a += 1
            plan.append(d)
    return plan, col_a, col_p


_PLAN, ACT_COLS, POOL_COLS = _chunk_plan()
LAST_RESULT = None                 # BassKernelResults of the most recent run
_PROGRAM = None                    # cached compiled Bass module


def _build_program():
    import concourse.bacc as bacc
    import concourse.mybir as mybir
    import concourse.tile as tile

    f32 = mybir.dt.float32
    bf16 = mybir.dt.bfloat16
    nc = bacc.Bacc("TRN2", target_bir_lowering=False, debug=False)

    p_in = nc.dram_tensor(
        "predictions", [B_PER_CORE, T, S, DIM], bf16, kind="ExternalInput"
    ).ap()
    g_in = nc.dram_tensor(
        "ground_truths", [B_PER_CORE, T, S, DIM], bf16, kind="ExternalInput"
    ).ap()
    out_act = nc.dram_tensor(
        "out_act", [P, ACT_COLS], f32, kind="ExternalOutput"
    ).ap()
    out_pool = nc.dram_tensor(
        "out_pool", [1, POOL_COLS], f32, kind="ExternalOutput"
    ).ap()

    umax = max(max(s) for s in _SCHED)

    with tile.TileContext(nc) as tc:
        with (
            tc.tile_pool(name="io", bufs=2) as io_pool,
            tc.tile_pool(name="scr", bufs=2) as scr_pool,
            tc.tile_pool(name="dummy", bufs=1) as dummy_pool,
            tc.tile_pool(name="cst", bufs=1) as cst_pool,
        ):
            acc_act = cst_pool.tile([P, ACT_COLS], f32, tag="acc_act")
            acc_pool_sums = cst_pool.tile([1, POOL_COLS], f32, tag="acc_pool")

            prev_b = -1
            t0 = 0
            for ch in _PLAN:
                b, u = ch["b"], ch["u"]
                if b != prev_b:
                    prev_b = b
                    t0 = 0
                rows = P * u
                n = u * DIM
                pc = p_in[b, t0 : t0 + rows].rearrange("(p u) s d -> p u s d", p=P)
                gc = g_in[b, t0 : t0 + rows].rearrange("(p u) s d -> p u s d", p=P)
                t0 += rows

                pt = io_pool.tile([P, umax * ROW], bf16, tag="pt")
                gt = io_pool.tile([P, umax * ROW], bf16, tag="gt")
                pv = pt[:, : u * ROW].rearrange("p (u s d) -> p u s d", u=u, s=S)
                gv = gt[:, : u * ROW].rearrange("p (u s d) -> p u s d", u=u, s=S)
                # per-source pieces p0,g0,p1,g1,p2,g2 so compute starts on the
                # first sources while later ones stream
                for s in range(S):
                    nc.sync.dma_start(pv[:, :, s, :], pc[:, :, s, :])
                    nc.sync.dma_start(gv[:, :, s, :], gc[:, :, s, :])

                # per-queue emission avoids head-of-line blocking:
                #   Pool queue: p3 subs (DMA-gated only) then p2 reduces
                #   DVE queue : p2 sub+sq (feeds pool) then p1 subs
                #   ACT queue : p3 squares then p1 squares
                by_unlock = lambda t: max(2 * t[1], 2 * t[2] + 1)

                with tc.high_priority(offset=_FEED_PRIO):
                    d3_tiles = []
                    for j, (col, s, r) in enumerate(
                        sorted(ch["p3"], key=by_unlock)
                    ):
                        df = scr_pool.tile([P, umax * DIM], bf16, tag="d3")
                        dv = df[:, :n].rearrange("p (u d) -> p u d", u=u)
                        nc.gpsimd.tensor_tensor(
                            out=dv, in0=pv[:, :, s, :], in1=gv[:, :, r, :],
                            op=mybir.AluOpType.subtract,
                        )
                        d3_tiles.append((col, dv))

                    sq_tiles = []
                    for j, (col, s, r) in enumerate(
                        sorted(ch["p2"], key=by_unlock)
                    ):
                        df = dummy_pool.tile([P, umax * DIM], bf16, tag="d2")
                        dv = df[:, :n].rearrange("p (u d) -> p u d", u=u)
                        nc.vector.tensor_tensor(
                            out=dv, in0=pv[:, :, s, :], in1=gv[:, :, r, :],
                            op=mybir.AluOpType.subtract,
                        )
                        sq = scr_pool.tile(
                            [P, umax * DIM], bf16, tag=f"sq{j % 2}"
                        )
                        sv = sq[:, :n].rearrange("p (u d) -> p u d", u=u)
                        nc.vector.tensor_tensor(
                            out=sv, in0=dv, in1=dv, op=mybir.AluOpType.mult,
                        )
                        sq_tiles.append((col, sq))

                    for col, sq in sq_tiles:
                        nc.gpsimd.reduce_sum(
                            acc_pool_sums[:, col : col + 1],
                            sq[:, :n],
                            axis=mybir.AxisListType.XYZWC,
                        )

                    for col, dv in d3_tiles:
                        nc.scalar.activation(
                            out=dv,
                            in_=dv,
                            func=mybir.ActivationFunctionType.Square,
                            accum_out=acc_act[:, col : col + 1],
                        )

                for j, (col, s, r) in enumerate(
                    sorted(ch["p1"], key=by_unlock)
                ):
                    df = scr_pool.tile([P, umax * DIM], bf16, tag=f"d1_{j % 3}")
                    dv = df[:, :n].rearrange("p (u d) -> p u d", u=u)
                    nc.vector.tensor_tensor(
                        out=dv, in0=pv[:, :, s, :], in1=gv[:, :, r, :],
                        op=mybir.AluOpType.subtract,
                    )
                    nc.scalar.activation(
                        out=dv,
                        in_=dv,
                        func=mybir.ActivationFunctionType.Square,
                        accum_out=acc_act[:, col : col + 1],
                    )

            nc.sync.dma_start(out_pool, acc_pool_sums[:])
            nc.sync.dma_start(out_act, acc_act[:])
    nc.compile()
    return nc


def _gather(results):
    d2 = np.zeros((B, S, S), dtype=np.float64)
    for c in range(N_CORES):
        oa = np.asarray(results[c]["out_act"], dtype=np.float64).sum(axis=0)
        op = np.asarray(results[c]["out_pool"], dtype=np.float64)[0]
        lo = c * B_PER_CORE
        for ch in _PLAN:
            bb = lo + ch["b"]
            for col, s, r in ch["p1"]:
                d2[bb, s, r] += oa[col]
            for col, s, r in ch["p3"]:
                d2[bb, s, r] += oa[col]
            for col, s, r in ch["p2"]:
                d2[bb, s, r] += op[col]
    return d2


def kernel(predictions: np.ndarray, ground_truths: np.ndarray) -> np.ndarray:
    global LAST_RESULT, _PROGRAM
    import ml_dtypes
    from concourse.bass_utils import run_bass_kernel_spmd

    if _PROGRAM is None:
        _PROGRAM = _build_program()
    nc = _PROGRAM

    preds = np.ascontiguousarray(
        np.asarray(predictions, dtype=np.float32).astype(ml_dtypes.bfloat16)
    )
    gts = np.ascontiguousarray(
        np.asarray(ground_truths, dtype=np.float32).astype(ml_dtypes.bfloat16)
    )

    in_maps = []
    for c in range(N_CORES):
        lo, hi = c * B_PER_CORE, (c + 1) * B_PER_CORE
        in_maps.append(
            {"predictions": preds[lo:hi], "ground_truths": gts[lo:hi]}
        )

    # retries: transient NRT/axon hiccups (e.g. a previously wedged core)
    # have been observed to clear on the next attempt
    last_exc = None
    for attempt in range(3):
        try:
            res = run_bass_kernel_spmd(nc, in_maps, list(range(N_CORES)))
            break
        except Exception as exc:   # noqa: BLE001
            last_exc = exc
            import time as _time

            _time.sleep(2.0 * (attempt + 1))
    else:
        raise last_exc
    LAST_RESULT = res

    d2 = _gather(res.results)
    D = np.sqrt(np.maximum(d2, 0.0))              # [B, S, S]
    dists = D[:, np.arange(S)[None, :], PERMS3]   # [B, 6, S]
    sum_ = dists.sum(axis=-1) / S                 # [B, 6]
    loss_per_perm = np.abs(sum_).mean(axis=0)     # [6]
    return np.array(np.log(loss_per_perm.min()), dtype=np.float32)


# revision 39
# speedup vs baseline: 1.0008x; 1.0008x over previous
"""Trainium2 Bass kernel for nn_MinLoss_69707319214519.

Computes log(min_p mean_b |sum_s D[b,s,perm[p,s]]/3|) where
D[b,s,r] = ||P[b,:,s,:] - G[b,:,r,:]||_F over (seq, dim).

Strategy (8 cores, 2 batches/core, bf16 streaming, compute-balanced):
  Inputs are cast to bf16 on the host (free), halving HBM traffic; the
  cost model then makes the three compute engines the bottleneck.  The
  squared distances are accumulated DIRECTLY as D2[s,r] = sum (P_s-G_r)^2
  (no Gram decomposition), so each chunk needs only the 9 (s,r) pairs.
  Each pair takes one of three balanced paths:
    p1 : DVE tensor_tensor subtract (2x bf16 mode) -> ACT Square-accum
    p2 : DVE subtract + DVE self-mult (both 2x)    -> Pool XYZWC reduce
    p3 : Pool subtract                             -> ACT Square-accum
  Chunk sizes ramp [2,2,4,8,...] and every chunk streams per-source
  (p0,g0,p1,g1,p2,g2) with ops emitted in data-unlock order, so all
  engines start within ~5us. Path counts per chunk come from an LP that
  equalizes engine END times (pipeline lags included), error-diffused
  to integers.  Host: gather partial sums -> D -> perm sums -> log(min).
"""

import numpy as np

B = 16
T = 4096
S = 3
DIM = 512
N_CORES = 8
B_PER_CORE = B // N_CORES          # 2
P = 128                            # SBUF partitions
ROW = S * DIM                      # 1536

# per-batch chunk schedule (units of P seq rows); each batch sums to 32.
_SCHED = [[2, 2, 4, 8, 8, 8], [8, 8, 8, 8]]

CROSS = [(s, r) for s in range(S) for r in range(S)]

PERMS3 = np.array(
    [[0, 1, 2], [0, 2, 1], [1, 0, 2], [1, 2, 0], [2, 0, 1], [2, 1, 0]]
)


def _op_costs(n):
    """Cost-model engine-busy ns per op of free-size n (bf16, calibrated
    against TimelineSim traces of this kernel)."""
    tt = 61 + 0.5208 * n             # DVE tensor_tensor (2x_1p mode)
    act = 559 + 0.8333 * n           # ACT activation + 187ns accum read
    pool_red = 95 + 1.3889 * n       # Pool reduce_sum XYZWC
    pool_tt = 95 + 2.0700 * n        # Pool tensor_tensor
    return tt, act, pool_red, pool_tt


# engine pipeline start lags (ns, whole program), tuned on traces
_LAG_DVE = 5400.0
_LAG_ACT = 7000.0
_LAG_POOL = 9000.0
# last chunks carry no p2 (pool reduces are second-order consumers and
# would gate the program end on the DVE->pool chain)
_TAIL_NORED = 0
# optional explicit per-chunk (y, z) override, list of pairs or None
_YZ_OVERRIDE = None
# scheduler priority boost for the pool-feeding ops (p2/p3 chains), in
# program-order units; lets next-chunk feeds outrank p1 backlog
_FEED_PRIO = 15


def _lp_targets(n, nchunks):
    """Fractional (p1, p2, p3) pair counts equalizing engine END times."""
    tt, act, pr, pt = _op_costs(n)
    la_d = _LAG_DVE / nchunks
    la_a = _LAG_ACT / nchunks
    la_p = _LAG_POOL / nchunks
    # D = tt x + 2 tt y + la_d ; A = act (x+z) + la_a ; P = pr y + pt z + la_p
    mat = np.array(
        [
            [tt - act, 2 * tt, -act],
            [act, -pr, act - pt],
            [1.0, 1.0, 1.0],
        ]
    )
    rhs = np.array([la_a - la_d, la_p - la_a, 9.0])
    try:
        x, y, z = np.linalg.solve(mat, rhs)
    except np.linalg.LinAlgError:
        x, y, z = 6.0, 2.0, 1.0
    x, y, z = max(x, 0.0), max(y, 0.0), max(z, 0.0)
    scale = 9.0 / (x + y + z)
    return x * scale, y * scale, z * scale


def _chunk_plan():
    """Per-chunk path assignments with error-diffused integer counts.

    Returns list of dicts: b, u, p1[(col,s,r)], p2[(col,s,r)], p3[(col,s,r)]
    plus global column totals (ACT cols for p1/p3, Pool cols for p2).
    """
    plan = []
    col_a = col_p = 0
    acc_y = acc_z = 0.0
    rot = 0
    nchunks = sum(len(s) for s in _SCHED)
    ci = 0
    for b, sched in enumerate(_SCHED):
        for u in sched:
            n = u * DIM
            _, y_t, z_t = _lp_targets(n, nchunks)
            acc_y += y_t
            y_i = int(np.floor(acc_y + 0.5))
            acc_y -= y_i
            acc_z += z_t
            z_i = int(np.floor(acc_z + 0.5))
            acc_z -= z_i
            y_i = min(y_i, 9)
            z_i = min(z_i, 9 - y_i)
            if ci >= nchunks - _TAIL_NORED:
                y_i = 0          # tail p2 pairs fall through to p1
            if _YZ_OVERRIDE is not None:
                y_i, z_i = _YZ_OVERRIDE[ci]
            ci += 1
            # earliest-unlocking pairs (data arrives per source
            # p0,g0,p1,g1,p2,g2) go to the pool-queue-independent p3 subs,
            # then to the p2 feed chain, so no engine head-of-line blocks
            pairs = CROSS[rot:] + CROSS[:rot]
            pairs = sorted(pairs, key=lambda sr: max(2 * sr[0], 2 * sr[1] + 1))
            rot = (rot + 2) % 9
            d = dict(b=b, u=u, p1=[], p2=[], p3=[])
            for s, r in pairs[:z_i]:
                d["p3"].append((col_a, s, r))
                col_a += 1
            for s, r in pairs[z_i : z_i + y_i]:
                d["p2"].append((col_p, s, r))
                col_p += 1
            for s, r in pairs[z_i + y_i :]:
                d["p1"].append((col_a, s, r))
                col_a += 1
            plan.append(d)
    return plan, col_a, col_p


_PLAN, ACT_COLS, POOL_COLS = _chunk_plan()
LAST_RESULT = None                 # BassKernelResults of the most recent run
_PROGRAM = None                    # cached compiled Bass module


def _build_program():
    import concourse.bacc as bacc
    import concourse.mybir as mybir
    import concourse.tile as tile

    f32 = mybir.dt.float32
    bf16 = mybir.dt.bfloat16
    nc = bacc.Bacc("TRN2", target_bir_lowering=False, debug=False)

    p_in = nc.dram_tensor(
        "predictions", [B_PER_CORE, T, S, DIM], bf16, kind="ExternalInput"
    ).ap()
    g_in = nc.dram_tensor(
        "ground_truths", [B_PER_CORE, T, S, DIM], bf16, kind="ExternalInput"
    ).ap()
    out_act = nc.dram_tensor(
        "out_act", [P, ACT_COLS], f32, kind="ExternalOutput"
    ).ap()
    out_pool = nc.dram_tensor(
        "out_pool", [1, POOL_COLS], f32, kind="ExternalOutput"
    ).ap()

    umax = max(max(s) for s in _SCHED)

    with tile.TileContext(nc) as tc:
        with (
            tc.tile_pool(name="io", bufs=2) as io_pool,
            tc.tile_pool(name="scr", bufs=2) as scr_pool,
            tc.tile_pool(name="dummy", bufs=1) as dummy_pool,
            tc.tile_pool(name="cst", bufs=1) as cst_pool,
        ):
            acc_act = cst_pool.tile([P, ACT_COLS], f32, tag="acc_act")
            acc_pool_sums = cst_pool.tile([1, POOL_COLS], f32, tag="acc_pool")

            prev_b = -1
            t0 = 0
            for ch in _PLAN:
                b, u = ch["b"], ch["u"]
                if b != prev_b:
                    prev_b = b
                    t0 = 0
                rows = P * u
                n = u * DIM
                pc = p_in[b, t0 : t0 + rows].rearrange("(p u) s d -> p u s d", p=P)
                gc = g_in[b, t0 : t0 + rows].rearrange("(p u) s d -> p u s d", p=P)
                t0 += rows

                pt = io_pool.tile([P, umax * ROW], bf16, tag="pt")
                gt = io_pool.tile([P, umax * ROW], bf16, tag="gt")
                pv = pt[:, : u * ROW].rearrange("p (u s d) -> p u s d", u=u, s=S)
                gv = gt[:, : u * ROW].rearrange("p (u s d) -> p u s d", u=u, s=S)
                # per-source pieces p0,g0,p1,g1,p2,g2 so compute starts on the
                # first sources while later ones stream
                for s in range(S):
                    nc.sync.dma_start(pv[:, :, s, :], pc[:, :, s, :])
                    nc.sync.dma_start(gv[:, :, s, :], gc[:, :, s, :])

                # per-queue emission avoids head-of-line blocking:
                #   Pool queue: p3 subs (DMA-gated only) then p2 reduces
                #   DVE queue : p2 sub+sq (feeds pool) then p1 subs
                #   ACT queue : p3 squares then p1 squares
                by_unlock = lambda t: max(2 * t[1], 2 * t[2] + 1)

                with tc.high_priority(offset=_FEED_PRIO):
                    d3_tiles = []
                    for j, (col, s, r) in enumerate(
                        sorted(ch["p3"], key=by_unlock)
                    ):
                        df = scr_pool.tile([P, umax * DIM], bf16, tag="d3")
                        dv = df[:, :n].rearrange("p (u d) -> p u d", u=u)
                        nc.gpsimd.tensor_tensor(
                            out=dv, in0=pv[:, :, s, :], in1=gv[:, :, r, :],
                            op=mybir.AluOpType.subtract,
                        )
                        d3_tiles.append((col, dv))

                    sq_tiles = []
                    for j, (col, s, r) in enumerate(
                        sorted(ch["p2"], key=by_unlock)
                    ):
                        df = dummy_pool.tile([P, umax * DIM], bf16, tag="d2")
                        dv = df[:, :n].rearrange("p (u d) -> p u d", u=u)
                        nc.vector.tensor_tensor(
                            out=dv, in0=pv[:, :, s, :], in1=gv[:, :, r, :],
                            op=mybir.AluOpType.subtract,
                        )
                        sq = scr_pool.tile(
                            [P, umax * DIM], bf16, tag=f"sq{j % 2}"
                        )
                        sv = sq[:, :n].rearrange("p (u d) -> p u d", u=u)
                        nc.vector.tensor_tensor(
                            out=sv, in0=dv, in1=dv, op=mybir.AluOpType.mult,
                        )
                        sq_tiles.append((col, sq))

                    for col, sq in sq_tiles:
                        nc.gpsimd.reduce_sum(
                            acc_pool_sums[:, col : col + 1],
                            sq[:, :n],
                            axis=mybir.AxisListType.XYZWC,
                        )

                    for col, dv in d3_tiles:
                        nc.scalar.activation(
                            out=dv,
                            in_=dv,
                            func=mybir.ActivationFunctionType.Square,
                            accum_out=acc_act[:, col : col + 1],
                        )

                for j, (col, s, r) in enumerate(
                    sorted(ch["p1"], key=by_unlock)
                ):
                    df = scr_pool.tile([P, umax * DIM], bf16, tag=f"d1_{j % 3}")
                    dv = df[:, :n].rearrange("p (u d) -> p u d", u=u)
                    nc.vector.tensor_tensor(
                        out=dv, in0=pv[:, :, s, :], in1=gv[:, :, r, :],
                        op=mybir.AluOpType.subtract,
                    )
                    nc.scalar.activation(
                        out=dv,
                        in_=dv,
                        func=mybir.ActivationFunctionType.Square,
                        accum_out=acc_act[:, col : col + 1],
                    )

            # bulk acc_act cols (all but the last chunk's) flush on the ACT
            # queue as soon as their writers finish, overlapping tail
            # compute; only the last chunk's few cols ride the critical
            # post-compute chain (tiny transfer)
            last_cols = [c for c, _, _ in _PLAN[-1]["p1"] + _PLAN[-1]["p3"]]
            b = min(last_cols) if last_cols else ACT_COLS
            if 0 < b < ACT_COLS:
                nc.scalar.dma_start(out_act[:, :b], acc_act[:, :b])
                nc.sync.dma_start(out_act[:, b:], acc_act[:, b:])
            else:
                nc.sync.dma_start(out_act, acc_act[:])
            nc.sync.dma_start(out_pool, acc_pool_sums[:])
    nc.compile()
    return nc


def _gather(results):
    d2 = np.zeros((B, S, S), dtype=np.float64)
    for c in range(N_CORES):
        oa = np.asarray(results[c]["out_act"], dtype=np.float64).sum(axis=0)
        op = np.asarray(results[c]["out_pool"], dtype=np.float64)[0]
        lo = c * B_PER_CORE
        for ch in _PLAN:
            bb = lo + ch["b"]
            for col, s, r in ch["p1"]:
                d2[bb, s, r] += oa[col]
            for col, s, r in ch["p3"]:
                d2[bb, s, r] += oa[col]
            for col, s, r in ch["p2"]:
                d2[bb, s, r] += op[col]
    return d2


def kernel(predictions: np.ndarray, ground_truths: np.ndarray) -> np.ndarray:
    global LAST_RESULT, _PROGRAM
    import ml_dtypes
    from concourse.bass_utils import run_bass_kernel_spmd

    if _PROGRAM is None:
        _PROGRAM = _build_program()
    nc = _PROGRAM

    preds = np.ascontiguousarray(
        np.asarray(predictions, dtype=np.float32).astype(ml_dtypes.bfloat16)
    )
    gts = np.ascontiguousarray(
        np.asarray(ground_truths, dtype=np.float32).astype(ml_dtypes.bfloat16)
    )

    in_maps = []
    for c in range(N_CORES):
        lo, hi = c * B_PER_CORE, (c + 1) * B_PER_CORE
        in_maps.append(
            {"predictions": preds[lo:hi], "ground_truths": gts[lo:hi]}
        )

    # retries: transient NRT/axon hiccups (e.g. a previously wedged core)
    # have been observed to clear on the next attempt
    last_exc = None
    for attempt in range(3):
        try:
            res = run_bass_kernel_spmd(nc, in_maps, list(range(N_CORES)))
            break
        except Exception as exc:   # noqa: BLE001
            last_exc = exc
            import time as _time

            _time.sleep(2.0 * (attempt + 1))
    else:
        raise last_exc
    LAST_RESULT = res

    d2 = _gather(res.results)
    D = np.sqrt(np.maximum(d2, 0.0))              # [B, S, S]
    dists = D[:, np.arange(S)[None, :], PERMS3]   # [B, 6, S]
    sum_ = dists.sum(axis=-1) / S                 # [B, 6]
    loss_per_perm = np.abs(sum_).mean(axis=0)     # [6]
    return np.array(np.log(loss_per_perm.min()), dtype=np.float32)
